# revision 68
# baseline (speedup 1.0000x reference)
"""Trainium2 Bass kernel for the Griffin-style gated linear recurrence.

Model (matching the jax reference, including its chunked-scan numerics):
    a = sigmoid(x @ Wa.T + decay_bias)
    i = sigmoid(x @ Wi.T)
    v = x @ Wv.T
    w = sqrt(max(1 - a*a, 1e-8)) * i * v
    chunked scan (chunk=64): cum_decay = prod of a within chunk;
    weighted = w / max(cum_decay, 1e-10); intra = cum_decay * cumsum(weighted);
    states = intra + cum_decay * carry.

This is algebraically the single global recurrence
    h[t] = a[t] * h[t-1] + min(1, cd[t]*1e10) * w[t]
with cd the within-chunk running product of a.  Here it is computed as
    u'  = (i * 1e10) * v                       (scalar_tensor_tensor)
    w'  = r * u',  r = sqrt(1 - a^2)           (ACT sqrt + Pool mult)
    cd  = scan: state = max(a*state, d1m)      (d1m = a at chunk starts)
    gw  = min(cd, 1e-10) * w'                  (scalar_tensor_tensor)
    h   = scan: state = a*state + gw
The max-reset works because cd in (0,1] and a <= 1; folding 1e10 into u'
makes the clamp a single fused min+mult.

Sharding: 4 batches x 2 channel-halves = 8 cores, no communication.
Per core 192 channels; weights are concatenated into 5 x 128-row matmul
passes (P0=Wa-lo, P1=Wi-lo, P2=Wv-lo, P3=[Wi-hi;Wa-hi], P4=[Wv-hi;pad]);
the hi-group decay sigmoid runs in place at partitions 64-127 and its
output is DMA-shifted to partitions 0-63 so the hi group stays aligned.
x and weights are bf16 (halves HBM traffic); PSUM and all vector-engine
tensors are fp32 (bf16/mixed scans and Pool casts measure much slower).

The sqrt(1-a^2) ops would thrash the ACT table against the sigmoids
(sqrt and sigmoid live in different activation-table sets, ~1.3us per
reload, and the Tile scheduler dispatches greedily), so sqrt phases are
*gated*: each flush's activations take their bias (the 1.0 in
sqrt(1 - m)) from a [P,1] tile produced by a Pool compare (a>=a) on the
trigger block's output, making the whole sqrt batch data-dependent on
pipeline progress.  PSUM-consuming ops (u') are emitted at high priority
so flush bursts never starve the PE of PSUM banks.
"""

import sys

if "/opt/trn_rl_repo" not in sys.path:
    sys.path.insert(0, "/opt/trn_rl_repo")

from contextlib import ExitStack, contextmanager


@contextmanager
def _null_ctx():
    yield

import ml_dtypes
import numpy as np

from concourse import bacc, mybir, tile
from concourse.bass_utils import run_bass_kernel_spmd

B, S = 4, 4096
DM, DR = 1024, 384
DC = DR // 2          # channels per core (192)
CH = 64               # scan chunk size
SB = 512              # sequence block per pipeline step
NB = S // SB
KT = DM // 128        # contraction tiles
NP = 5                # matmul passes per block (640 = 5*128 rows)
WROWS = NP * 128

F32 = mybir.dt.float32
BF16 = mybir.dt.bfloat16
AFT = mybir.ActivationFunctionType
OP = mybir.AluOpType

PASS_ORDER = (3, 0, 1, 4, 2)

# blocks after whose section the pending blocks (excluding that block)
# are flushed through the sqrt phase
FLUSH_AT = (1, 3, 5, 7)

_CACHED_NC = None


def _build_nc():
    nc = bacc.Bacc(trn_type="TRN2")

    # host-swizzled layouts: xt is [128, NB*KT*SB] with per-(partition,
    # block) contiguous 8KB runs, wcat is [128, NP*KT*128] with per-pass
    # contiguous 2KB runs — few large DMA descriptors instead of ~1K tiny
    # ones per transfer.
    xT = nc.dram_tensor("xt", [128, NB * KT * SB], BF16, kind="ExternalInput")
    wT = nc.dram_tensor("wcat", [128, NP * KT * 128], BF16,
                        kind="ExternalInput")
    bias0 = nc.dram_tensor("bias0", [128, 1], F32, kind="ExternalInput")
    bias1 = nc.dram_tensor("bias1", [64, 1], F32, kind="ExternalInput")
    out = nc.dram_tensor("out", [DC, S], F32, kind="ExternalOutput")

    with tile.TileContext(nc) as tc, ExitStack() as ctx:
        wp = ctx.enter_context(tc.tile_pool(name="wp", bufs=1))
        cp = ctx.enter_context(tc.tile_pool(name="cp", bufs=1))
        xp = ctx.enter_context(tc.tile_pool(name="xp", bufs=2))
        pp = ctx.enter_context(tc.tile_pool(name="pp", bufs=1, space="PSUM"))
        shp = ctx.enter_context(tc.tile_pool(name="shp", bufs=2))
        # pools whose tiles survive until the sqrt flush.  Depth matters:
        # if a pool hits capacity in the compile-time scheduler's
        # simulation, it slots unrelated work (sigmoids) into the sqrt
        # bursts and thrashes the activation table.
        ap = ctx.enter_context(tc.tile_pool(name="ap", bufs=8))
        up = ctx.enter_context(tc.tile_pool(name="up", bufs=8))
        cdp = ctx.enter_context(tc.tile_pool(name="cdp", bufs=6))
        rp = ctx.enter_context(tc.tile_pool(name="rp", bufs=8))
        mp = ctx.enter_context(tc.tile_pool(name="mp", bufs=8))
        # short-lived per-block tiles
        ip = ctx.enter_context(tc.tile_pool(name="ip", bufs=2))
        wwp = ctx.enter_context(tc.tile_pool(name="wwp", bufs=2))
        gwp = ctx.enter_context(tc.tile_pool(name="gwp", bufs=2))
        hp = ctx.enter_context(tc.tile_pool(name="hp", bufs=2))
        gtp = ctx.enter_context(tc.tile_pool(name="gtp", bufs=2))

        # --- constants -------------------------------------------------
        # weights split per pass so the first matmul only waits on its
        # slice; emission of the actual DMAs happens after block 0's x DMA
        # so x wins the sync-queue dispatch race.
        w_sb = wp.tile([128, NP, KT, 128], BF16, tag="w")

        b0 = cp.tile([128, 1], F32, tag="b0")
        nc.scalar.dma_start(b0[:], bias0[:, :])
        # hi-group bias lives at partitions 64-127 to match its PSUM input
        b1 = cp.tile([128, 1], F32, tag="b1")
        nc.scalar.dma_start(b1[64:128, :], bias1[:, :])

        # persistent d1m tiles: zero except chunk-start columns, refreshed
        # with a[:, ::64] every block.
        d1m0 = cp.tile([128, SB], F32, tag="d1m0")
        nc.vector.memset(d1m0[:], 0.0)
        d1m1 = cp.tile([64, SB], F32, tag="d1m1")
        nc.vector.memset(d1m1[:], 0.0)

        # --- main pipeline over sequence blocks ------------------------
        pend = []
        state = {"init0": 0.0, "init1": 0.0}

        def flush(blocks, last=False):
            """Run the sqrt phase + w/gw/h for `blocks`.  The sqrt batch
            is gated (via its bias tile) on the last flushed block's u
            tiles — i.e. it becomes ready exactly when the flush's own
            inputs are complete — and runs at high priority, so the whole
            batch executes contiguously (one activation-table round trip)
            without the greedy scheduler interleaving sigmoids."""
            if not blocks:
                return
            # m = a*a is produced in the block body (spread Pool load).
            # The gate ties the sqrt batch to the LAST flushed block's m:
            # Pool executes in order, so m_last done implies every m in
            # the batch is done — all sqrts become ready simultaneously
            # and run contiguously (one activation-table round trip).
            if last:
                # nothing follows the final sqrt batch on ACT, so no
                # gating is needed — let it run the moment m lands
                bias0_ap = bias1_ap = 1.0
            else:
                lm0, lm1 = blocks[-1][7], blocks[-1][8]
                gate = gtp.tile([128, 1], F32, tag="gate")
                with tc.high_priority():
                    nc.vector.tensor_tensor(
                        gate[0:64, :], lm1[:, 0:1], lm1[:, 0:1],
                        op=OP.is_ge)
                    nc.vector.tensor_tensor(
                        gate[64:128, :], lm0[64:128, 0:1],
                        lm0[64:128, 0:1], op=OP.is_ge)
                bias0_ap, bias1_ap = gate[:], gate[0:64, :]
            rs = []
            with tc.high_priority():
                for (jb, aj0, aj1, uj0, uj1, cj0, cj1, m0, m1) in blocks:
                    r0 = rp.tile([128, SB], BF16, tag="r0")
                    nc.scalar.activation(
                        r0[:], m0[:], AFT.Sqrt, bias=bias0_ap, scale=-1.0)
                    r1 = rp.tile([64, SB], BF16, tag="r1")
                    nc.scalar.activation(
                        r1[:], m1[:], AFT.Sqrt, bias=bias1_ap, scale=-1.0)
                    rs.append((r0, r1))
            for (jb, aj0, aj1, uj0, uj1, cj0, cj1, m0, m1), (r0, r1) \
                    in zip(blocks, rs):
                sj = jb * SB
                # w0 on DVE (bf16 2x mode, ~360ns) in parallel with w1 on
                # Pool — halves the serial w chain in the flush bursts
                w0 = wwp.tile([128, SB], BF16, tag="w0")
                nc.vector.tensor_mul(w0[:], r0[:], uj0[:])
                w1 = wwp.tile([64, SB], BF16, tag="w1")
                nc.gpsimd.tensor_mul(w1[:], r1[:], uj1[:])
                # gw = min(cd, 1e-10) * w'   (the 1e10 is folded into u')
                gw0 = gwp.tile([128, SB], F32, tag="gw0")
                nc.vector.scalar_tensor_tensor(
                    gw0[:], cj0[:], 1e-10, w0[:], op0=OP.min, op1=OP.mult)
                gw1 = gwp.tile([64, SB], F32, tag="gw1")
                nc.vector.scalar_tensor_tensor(
                    gw1[:], cj1[:], 1e-10, w1[:], op0=OP.min, op1=OP.mult)
                h0 = hp.tile([128, SB], F32, tag="h0")
                nc.vector.tensor_tensor_scan(
                    h0[:], aj0[:], gw0[:], state["init0"],
                    op0=OP.mult, op1=OP.add)
                h1 = hp.tile([64, SB], F32, tag="h1")
                nc.vector.tensor_tensor_scan(
                    h1[:], aj1[:], gw1[:], state["init1"],
                    op0=OP.mult, op1=OP.add)
                # round-robin output DMAs over three engine queues so the
                # 3.1MB of h traffic drains in parallel instead of
                # serializing (~11us) on the sync queue at the end; the
                # final block's pair is split across two queues
                if last and jb == blocks[-1][0]:
                    nc.sync.dma_start(out[0:128, sj:sj + SB], h0[:])
                    nc.scalar.dma_start(out[128:DC, sj:sj + SB], h1[:])
                else:
                    out_dma = (nc.sync.dma_start, nc.scalar.dma_start,
                               nc.gpsimd.dma_start)[jb % 3]
                    out_dma(out[0:128, sj:sj + SB], h0[:])
                    out_dma(out[128:DC, sj:sj + SB], h1[:])
                state["init0"] = h0[:, SB - 1:SB]
                state["init1"] = h1[:, SB - 1:SB]

        x_src = xT.rearrange("p (nb k s) -> p nb k s", nb=NB, k=KT)
        w_src = wT.rearrange("p (np k c) -> p np k c", np=NP, k=KT)

        for ib in range(NB):
            s0 = ib * SB

            x_sb = xp.tile([128, KT, SB], BF16, tag="x")
            if ib == 0:
                # startup is aggregate-DMA-bandwidth-bound: sequence the
                # transfers so only w-P3 + x[k0] (384KB) gate the first
                # matmul and the rest streams in behind it
                with tc.high_priority():
                    nc.scalar.dma_start(w_sb[:, 3], w_src[:, 3])
                    for k in range(0, KT, 2):
                        nc.sync.dma_start(
                            x_sb[:, k:k + 1], x_src[:, ib, k:k + 1])
                    for k in range(1, KT, 2):
                        nc.scalar.dma_start(
                            x_sb[:, k:k + 1], x_src[:, ib, k:k + 1])
                for p in (0, 1, 4, 2):
                    nc.gpsimd.dma_start(w_sb[:, p], w_src[:, p])
            else:
                # two-queue split: a 1MB block takes ~9us on one queue,
                # leaving no slack against the ~10us block period
                nc.sync.dma_start(x_sb[:, 0:4], x_src[:, ib, 0:4])
                nc.scalar.dma_start(x_sb[:, 4:8], x_src[:, ib, 4:8])

            z = {}
            for p in PASS_ORDER:
                zt = pp.tile([128, SB], F32, tag=f"z{p}")
                for k in range(KT):
                    nc.tensor.matmul(
                        zt[:],
                        w_sb[:, p, k, :],
                        x_sb[:, k, :],
                        start=(k == 0),
                        stop=(k == KT - 1),
                    )
                z[p] = zt

            # ---- ACT sigmoids (single table set); PSUM consumers run at
            # high priority so flush bursts never stall the PE ----
            a1hi = shp.tile([128, SB], F32, tag="a1hi")
            a1 = ap.tile([64, SB], F32, tag="a1")
            a0 = ap.tile([128, SB], F32, tag="a0")
            i0 = ip.tile([128, SB], BF16, tag="i0")
            i1 = ip.tile([64, SB], BF16, tag="i1")
            u0 = up.tile([128, SB], BF16, tag="u0")
            u1 = up.tile([64, SB], BF16, tag="u1")
            nc.scalar.activation(
                a1hi[64:128, :], z[3][64:128, :], AFT.Sigmoid,
                bias=b1[64:128, :])
            nc.sync.dma_start(a1[:], a1hi[64:128, :])
            nc.scalar.activation(a0[:], z[0][:], AFT.Sigmoid, bias=b0[:])
            nc.scalar.activation(i0[:], z[1][:], AFT.Sigmoid)
            nc.scalar.activation(i1[:], z[3][0:64, :], AFT.Sigmoid)
            with tc.high_priority():
                # u' = (i * 1e10) * v
                nc.vector.scalar_tensor_tensor(
                    u0[:], i0[:], 1e10, z[2][:], op0=OP.mult, op1=OP.mult)
                nc.vector.scalar_tensor_tensor(
                    u1[:], i1[:], 1e10, z[4][0:64, :],
                    op0=OP.mult, op1=OP.mult)

            # ---- Pool: d1m refresh + m = a*a ----
            nc.vector.tensor_copy(d1m0[:, 0:SB:CH], a0[:, 0:SB:CH])
            nc.vector.tensor_copy(d1m1[:, 0:SB:CH], a1[:, 0:SB:CH])
            m0 = mp.tile([128, SB], F32, tag="m0")
            m1 = mp.tile([64, SB], F32, tag="m1")
            with tc.high_priority():
                if ib == NB - 1:
                    # last block: m on DVE so the tail-critical sqrt
                    # doesn't queue behind flush w-ops on Pool
                    nc.vector.tensor_mul(m0[:], a0[:], a0[:])
                    nc.vector.tensor_mul(m1[:], a1[:], a1[:])
                else:
                    nc.gpsimd.tensor_mul(m0[:], a0[:], a0[:])
                    nc.gpsimd.tensor_mul(m1[:], a1[:], a1[:])

            # ---- cd scan with max-reset (DVE) ----
            cd0 = cdp.tile([128, SB], F32, tag="cd0")
            nc.vector.tensor_tensor_scan(
                cd0[:], a0[:], d1m0[:], 0.0, op0=OP.mult, op1=OP.max)
            cd1 = cdp.tile([64, SB], F32, tag="cd1")
            nc.vector.tensor_tensor_scan(
                cd1[:], a1[:], d1m1[:], 0.0, op0=OP.mult, op1=OP.max)

            pend.append((ib, a0, a1, u0, u1, cd0, cd1, m0, m1))

            if ib in FLUSH_AT:
                flush(pend[:-1])
                pend = pend[-1:]

        flush(pend, last=True)

    nc.finalize()
    return nc


def _make_in_maps(x, Wa, Wi, Wv, decay_bias):
    x = np.asarray(x, dtype=np.float32)
    Wa = np.asarray(Wa, dtype=np.float32)
    Wi = np.asarray(Wi, dtype=np.float32)
    Wv = np.asarray(Wv, dtype=np.float32)
    decay_bias = np.asarray(decay_bias, dtype=np.float32)

    in_maps = []
    for b in range(B):
        # xt_sw[p, ib, k, s'] = x[b, ib*SB+s', k*128+p]
        xt_sw = np.ascontiguousarray(
            x[b].reshape(NB, SB, KT, 128).transpose(3, 0, 2, 1)
            .reshape(128, NB * KT * SB)).astype(ml_dtypes.bfloat16)
        for j in range(2):
            c0 = j * DC
            wcat = np.concatenate([
                Wa[c0:c0 + 128],
                Wi[c0:c0 + 128],
                Wv[c0:c0 + 128],
                Wi[c0 + 128:c0 + 192],
                Wa[c0 + 128:c0 + 192],
                Wv[c0 + 128:c0 + 192],
                np.zeros((64, DM), np.float32),
            ])                                   # [640, DM]
            # w_sw[p, pi, k, c] = wcat[pi*128 + c, k*128 + p]
            w_sw = np.ascontiguousarray(
                wcat.reshape(NP, 128, KT, 128).transpose(3, 0, 2, 1)
                .reshape(128, NP * KT * 128)).astype(ml_dtypes.bfloat16)
            in_maps.append({
                "xt": xt_sw,
                "wcat": w_sw,
                "bias0": np.ascontiguousarray(
                    decay_bias[c0:c0 + 128, None]),
                "bias1": np.ascontiguousarray(
                    decay_bias[c0 + 128:c0 + 192, None]),
            })
    return in_maps


def kernel(x, Wa, Wi, Wv, decay_bias):
    global _CACHED_NC
    if _CACHED_NC is None:
        _CACHED_NC = _build_nc()
    nc = _CACHED_NC

    in_maps = _make_in_maps(x, Wa, Wi, Wv, decay_bias)
    res = run_bass_kernel_spmd(nc, in_maps, core_ids=list(range(8)))

    out = np.empty((B, S, DR), dtype=np.float32)
    for b in range(B):
        for j in range(2):
            core = 2 * b + j
            out[b, :, j * DC:(j + 1) * DC] = res.results[core]["out"].T
    return out


# revision 70
# speedup vs baseline: 1.0249x; 1.0249x over previous
"""Trainium2 Bass kernel for the Griffin-style gated linear recurrence.

Model (matching the jax reference, including its chunked-scan numerics):
    a = sigmoid(x @ Wa.T + decay_bias)
    i = sigmoid(x @ Wi.T)
    v = x @ Wv.T
    w = sqrt(max(1 - a*a, 1e-8)) * i * v
    chunked scan (chunk=64): cum_decay = prod of a within chunk;
    weighted = w / max(cum_decay, 1e-10); intra = cum_decay * cumsum(weighted);
    states = intra + cum_decay * carry.

This is algebraically the single global recurrence
    h[t] = a[t] * h[t-1] + min(1, cd[t]*1e10) * w[t]
with cd the within-chunk running product of a.  Here it is computed as
    u'  = (i * 1e10) * v                       (scalar_tensor_tensor)
    w'  = r * u',  r = sqrt(1 - a^2)           (ACT sqrt + Pool mult)
    cd  = scan: state = max(a*state, d1m)      (d1m = a at chunk starts)
    gw  = min(cd, 1e-10) * w'                  (scalar_tensor_tensor)
    h   = scan: state = a*state + gw
The max-reset works because cd in (0,1] and a <= 1; folding 1e10 into u'
makes the clamp a single fused min+mult.

Sharding: 4 batches x 2 channel-halves = 8 cores, no communication.
Per core 192 channels; weights are concatenated into 5 x 128-row matmul
passes (P0=Wa-lo, P1=Wi-lo, P2=Wv-lo, P3=[Wi-hi;Wa-hi], P4=[Wv-hi;pad]);
the hi-group decay sigmoid runs in place at partitions 64-127 and its
output is DMA-shifted to partitions 0-63 so the hi group stays aligned.
x and weights are bf16 (halves HBM traffic); PSUM and all vector-engine
tensors are fp32 (bf16/mixed scans and Pool casts measure much slower).

The sqrt(1-a^2) ops would thrash the ACT table against the sigmoids
(sqrt and sigmoid live in different activation-table sets, ~1.3us per
reload, and the Tile scheduler dispatches greedily), so sqrt phases are
*gated*: each flush's activations take their bias (the 1.0 in
sqrt(1 - m)) from a [P,1] tile produced by a Pool compare (a>=a) on the
trigger block's output, making the whole sqrt batch data-dependent on
pipeline progress.  PSUM-consuming ops (u') are emitted at high priority
so flush bursts never starve the PE of PSUM banks.
"""

import sys

if "/opt/trn_rl_repo" not in sys.path:
    sys.path.insert(0, "/opt/trn_rl_repo")

from contextlib import ExitStack, contextmanager


@contextmanager
def _null_ctx():
    yield

import ml_dtypes
import numpy as np

from concourse import bacc, mybir, tile
from concourse.bass_utils import run_bass_kernel_spmd

B, S = 4, 4096
DM, DR = 1024, 384
DC = DR // 2          # channels per core (192)
CH = 64               # scan chunk size
SB = 512              # sequence block per pipeline step
NB = S // SB
KT = DM // 128        # contraction tiles
NP = 5                # matmul passes per block (640 = 5*128 rows)
WROWS = NP * 128

F32 = mybir.dt.float32
BF16 = mybir.dt.bfloat16
AFT = mybir.ActivationFunctionType
OP = mybir.AluOpType

PASS_ORDER = (3, 0, 1, 4, 2)

# blocks after whose section the pending blocks (excluding that block)
# are flushed through the sqrt phase
FLUSH_AT = (1, 3, 5, 7)

_CACHED_NC = None


def _build_nc():
    nc = bacc.Bacc(trn_type="TRN2")

    # host-swizzled layouts: xt is [128, NB*KT*SB] with per-(partition,
    # block) contiguous 8KB runs, wcat is [128, NP*KT*128] with per-pass
    # contiguous 2KB runs — few large DMA descriptors instead of ~1K tiny
    # ones per transfer.
    xT = nc.dram_tensor("xt", [128, NB * KT * SB], BF16, kind="ExternalInput")
    wT = nc.dram_tensor("wcat", [128, NP * KT * 128], BF16,
                        kind="ExternalInput")
    bias0 = nc.dram_tensor("bias0", [128, 1], F32, kind="ExternalInput")
    bias1 = nc.dram_tensor("bias1", [64, 1], F32, kind="ExternalInput")
    out = nc.dram_tensor("out", [DC, S], F32, kind="ExternalOutput")

    with tile.TileContext(nc) as tc, ExitStack() as ctx:
        wp = ctx.enter_context(tc.tile_pool(name="wp", bufs=1))
        cp = ctx.enter_context(tc.tile_pool(name="cp", bufs=1))
        xp = ctx.enter_context(tc.tile_pool(name="xp", bufs=3))
        pp = ctx.enter_context(tc.tile_pool(name="pp", bufs=1, space="PSUM"))
        shp = ctx.enter_context(tc.tile_pool(name="shp", bufs=2))
        # pools whose tiles survive until the sqrt flush.  Depth matters:
        # if a pool hits capacity in the compile-time scheduler's
        # simulation, it slots unrelated work (sigmoids) into the sqrt
        # bursts and thrashes the activation table.
        ap = ctx.enter_context(tc.tile_pool(name="ap", bufs=8))
        up = ctx.enter_context(tc.tile_pool(name="up", bufs=8))
        cdp = ctx.enter_context(tc.tile_pool(name="cdp", bufs=6))
        rp = ctx.enter_context(tc.tile_pool(name="rp", bufs=8))
        mp = ctx.enter_context(tc.tile_pool(name="mp", bufs=8))
        # short-lived per-block tiles
        ip = ctx.enter_context(tc.tile_pool(name="ip", bufs=2))
        wwp = ctx.enter_context(tc.tile_pool(name="wwp", bufs=2))
        gwp = ctx.enter_context(tc.tile_pool(name="gwp", bufs=2))
        hp = ctx.enter_context(tc.tile_pool(name="hp", bufs=2))
        gtp = ctx.enter_context(tc.tile_pool(name="gtp", bufs=2))

        # --- constants -------------------------------------------------
        # weights split per pass so the first matmul only waits on its
        # slice; emission of the actual DMAs happens after block 0's x DMA
        # so x wins the sync-queue dispatch race.
        w_sb = wp.tile([128, NP, KT, 128], BF16, tag="w")

        b0 = cp.tile([128, 1], F32, tag="b0")
        nc.scalar.dma_start(b0[:], bias0[:, :])
        # hi-group bias lives at partitions 64-127 to match its PSUM input
        b1 = cp.tile([128, 1], F32, tag="b1")
        nc.scalar.dma_start(b1[64:128, :], bias1[:, :])

        # persistent d1m tiles: zero except chunk-start columns, refreshed
        # with a[:, ::64] every block.
        d1m0 = cp.tile([128, SB], F32, tag="d1m0")
        nc.vector.memset(d1m0[:], 0.0)
        d1m1 = cp.tile([64, SB], F32, tag="d1m1")
        nc.vector.memset(d1m1[:], 0.0)

        # --- main pipeline over sequence blocks ------------------------
        pend = []
        state = {"init0": 0.0, "init1": 0.0}

        def flush(blocks, last=False):
            """Run the sqrt phase + w/gw/h for `blocks`.  The sqrt batch
            is gated (via its bias tile) on the last flushed block's u
            tiles — i.e. it becomes ready exactly when the flush's own
            inputs are complete — and runs at high priority, so the whole
            batch executes contiguously (one activation-table round trip)
            without the greedy scheduler interleaving sigmoids."""
            if not blocks:
                return
            # m = a*a is produced in the block body (spread Pool load).
            # The gate ties the sqrt batch to the LAST flushed block's m:
            # Pool executes in order, so m_last done implies every m in
            # the batch is done — all sqrts become ready simultaneously
            # and run contiguously (one activation-table round trip).
            if last:
                # nothing follows the final sqrt batch on ACT, so no
                # gating is needed — let it run the moment m lands
                bias0_ap = bias1_ap = 1.0
            else:
                lm0, lm1 = blocks[-1][7], blocks[-1][8]
                gate = gtp.tile([128, 1], F32, tag="gate")
                with tc.high_priority():
                    nc.vector.tensor_tensor(
                        gate[0:64, :], lm1[:, 0:1], lm1[:, 0:1],
                        op=OP.is_ge)
                    nc.vector.tensor_tensor(
                        gate[64:128, :], lm0[64:128, 0:1],
                        lm0[64:128, 0:1], op=OP.is_ge)
                bias0_ap, bias1_ap = gate[:], gate[0:64, :]
            rs = []
            with tc.high_priority():
                for (jb, aj0, aj1, uj0, uj1, cj0, cj1, m0, m1) in blocks:
                    r0 = rp.tile([128, SB], BF16, tag="r0")
                    nc.scalar.activation(
                        r0[:], m0[:], AFT.Sqrt, bias=bias0_ap, scale=-1.0)
                    r1 = rp.tile([64, SB], BF16, tag="r1")
                    nc.scalar.activation(
                        r1[:], m1[:], AFT.Sqrt, bias=bias1_ap, scale=-1.0)
                    rs.append((r0, r1))
            for (jb, aj0, aj1, uj0, uj1, cj0, cj1, m0, m1), (r0, r1) \
                    in zip(blocks, rs):
                sj = jb * SB
                # w0 on DVE (bf16 2x mode, ~360ns) in parallel with w1 on
                # Pool — halves the serial w chain in the flush bursts
                w0 = wwp.tile([128, SB], BF16, tag="w0")
                nc.vector.tensor_mul(w0[:], r0[:], uj0[:])
                w1 = wwp.tile([64, SB], BF16, tag="w1")
                nc.gpsimd.tensor_mul(w1[:], r1[:], uj1[:])
                # gw = min(cd, 1e-10) * w'   (the 1e10 is folded into u')
                gw0 = gwp.tile([128, SB], F32, tag="gw0")
                nc.vector.scalar_tensor_tensor(
                    gw0[:], cj0[:], 1e-10, w0[:], op0=OP.min, op1=OP.mult)
                gw1 = gwp.tile([64, SB], F32, tag="gw1")
                nc.vector.scalar_tensor_tensor(
                    gw1[:], cj1[:], 1e-10, w1[:], op0=OP.min, op1=OP.mult)
                h0 = hp.tile([128, SB], F32, tag="h0")
                nc.vector.tensor_tensor_scan(
                    h0[:], aj0[:], gw0[:], state["init0"],
                    op0=OP.mult, op1=OP.add)
                h1 = hp.tile([64, SB], F32, tag="h1")
                nc.vector.tensor_tensor_scan(
                    h1[:], aj1[:], gw1[:], state["init1"],
                    op0=OP.mult, op1=OP.add)
                # round-robin output DMAs over three engine queues so the
                # 3.1MB of h traffic drains in parallel instead of
                # serializing (~11us) on the sync queue at the end; the
                # final block's pair is split across two queues
                if last and jb == blocks[-1][0]:
                    nc.sync.dma_start(out[0:128, sj:sj + SB], h0[:])
                    nc.scalar.dma_start(out[128:DC, sj:sj + SB], h1[:])
                else:
                    out_dma = (nc.sync.dma_start, nc.scalar.dma_start,
                               nc.gpsimd.dma_start)[jb % 3]
                    out_dma(out[0:128, sj:sj + SB], h0[:])
                    out_dma(out[128:DC, sj:sj + SB], h1[:])
                state["init0"] = h0[:, SB - 1:SB]
                state["init1"] = h1[:, SB - 1:SB]

        x_src = xT.rearrange("p (nb k s) -> p nb k s", nb=NB, k=KT)
        w_src = wT.rearrange("p (np k c) -> p np k c", np=NP, k=KT)

        for ib in range(NB):
            s0 = ib * SB

            x_sb = xp.tile([128, KT, SB], BF16, tag="x")
            if ib == 0:
                # startup is aggregate-DMA-bandwidth-bound: sequence the
                # transfers so only w-P3 + x[k0] (384KB) gate the first
                # matmul and the rest streams in behind it
                with tc.high_priority():
                    nc.scalar.dma_start(w_sb[:, 3], w_src[:, 3])
                    for k in range(0, KT, 2):
                        nc.sync.dma_start(
                            x_sb[:, k:k + 1], x_src[:, ib, k:k + 1])
                    for k in range(1, KT, 2):
                        nc.scalar.dma_start(
                            x_sb[:, k:k + 1], x_src[:, ib, k:k + 1])
                for p in (0, 1, 4, 2):
                    nc.gpsimd.dma_start(w_sb[:, p], w_src[:, p])
            else:
                # two-queue split: a 1MB block takes ~9us on one queue,
                # leaving no slack against the ~10us block period
                nc.sync.dma_start(x_sb[:, 0:4], x_src[:, ib, 0:4])
                nc.scalar.dma_start(x_sb[:, 4:8], x_src[:, ib, 4:8])

            z = {}
            for p in PASS_ORDER:
                zt = pp.tile([128, SB], F32, tag=f"z{p}")
                for k in range(KT):
                    nc.tensor.matmul(
                        zt[:],
                        w_sb[:, p, k, :],
                        x_sb[:, k, :],
                        start=(k == 0),
                        stop=(k == KT - 1),
                    )
                z[p] = zt

            # ---- ACT sigmoids (single table set); PSUM consumers run at
            # high priority so flush bursts never stall the PE ----
            a1hi = shp.tile([128, SB], F32, tag="a1hi")
            a1 = ap.tile([64, SB], F32, tag="a1")
            a0 = ap.tile([128, SB], F32, tag="a0")
            i0 = ip.tile([128, SB], BF16, tag="i0")
            i1 = ip.tile([64, SB], BF16, tag="i1")
            u0 = up.tile([128, SB], BF16, tag="u0")
            u1 = up.tile([64, SB], BF16, tag="u1")
            nc.scalar.activation(
                a1hi[64:128, :], z[3][64:128, :], AFT.Sigmoid,
                bias=b1[64:128, :])
            nc.sync.dma_start(a1[:], a1hi[64:128, :])
            nc.scalar.activation(a0[:], z[0][:], AFT.Sigmoid, bias=b0[:])
            nc.scalar.activation(i0[:], z[1][:], AFT.Sigmoid)
            nc.scalar.activation(i1[:], z[3][0:64, :], AFT.Sigmoid)
            with tc.high_priority():
                # u' = (i * 1e10) * v
                nc.vector.scalar_tensor_tensor(
                    u0[:], i0[:], 1e10, z[2][:], op0=OP.mult, op1=OP.mult)
                nc.vector.scalar_tensor_tensor(
                    u1[:], i1[:], 1e10, z[4][0:64, :],
                    op0=OP.mult, op1=OP.mult)

            # ---- Pool: d1m refresh + m = a*a ----
            nc.gpsimd.tensor_copy(d1m0[:, 0:SB:CH], a0[:, 0:SB:CH])
            nc.gpsimd.tensor_copy(d1m1[:, 0:SB:CH], a1[:, 0:SB:CH])
            m0 = mp.tile([128, SB], F32, tag="m0")
            m1 = mp.tile([64, SB], F32, tag="m1")
            with tc.high_priority():
                if ib == NB - 1:
                    # last block: m on DVE so the tail-critical sqrt
                    # doesn't queue behind flush w-ops on Pool
                    nc.vector.tensor_mul(m0[:], a0[:], a0[:])
                    nc.vector.tensor_mul(m1[:], a1[:], a1[:])
                else:
                    nc.gpsimd.tensor_mul(m0[:], a0[:], a0[:])
                    nc.gpsimd.tensor_mul(m1[:], a1[:], a1[:])

            # ---- cd scan with max-reset (DVE) ----
            cd0 = cdp.tile([128, SB], F32, tag="cd0")
            nc.vector.tensor_tensor_scan(
                cd0[:], a0[:], d1m0[:], 0.0, op0=OP.mult, op1=OP.max)
            cd1 = cdp.tile([64, SB], F32, tag="cd1")
            nc.vector.tensor_tensor_scan(
                cd1[:], a1[:], d1m1[:], 0.0, op0=OP.mult, op1=OP.max)

            pend.append((ib, a0, a1, u0, u1, cd0, cd1, m0, m1))

            if ib in FLUSH_AT:
                flush(pend[:-1])
                pend = pend[-1:]

        flush(pend, last=True)

    nc.finalize()
    return nc


def _make_in_maps(x, Wa, Wi, Wv, decay_bias):
    x = np.asarray(x, dtype=np.float32)
    Wa = np.asarray(Wa, dtype=np.float32)
    Wi = np.asarray(Wi, dtype=np.float32)
    Wv = np.asarray(Wv, dtype=np.float32)
    decay_bias = np.asarray(decay_bias, dtype=np.float32)

    in_maps = []
    for b in range(B):
        # xt_sw[p, ib, k, s'] = x[b, ib*SB+s', k*128+p]
        xt_sw = np.ascontiguousarray(
            x[b].reshape(NB, SB, KT, 128).transpose(3, 0, 2, 1)
            .reshape(128, NB * KT * SB)).astype(ml_dtypes.bfloat16)
        for j in range(2):
            c0 = j * DC
            wcat = np.concatenate([
                Wa[c0:c0 + 128],
                Wi[c0:c0 + 128],
                Wv[c0:c0 + 128],
                Wi[c0 + 128:c0 + 192],
                Wa[c0 + 128:c0 + 192],
                Wv[c0 + 128:c0 + 192],
                np.zeros((64, DM), np.float32),
            ])                                   # [640, DM]
            # w_sw[p, pi, k, c] = wcat[pi*128 + c, k*128 + p]
            w_sw = np.ascontiguousarray(
                wcat.reshape(NP, 128, KT, 128).transpose(3, 0, 2, 1)
                .reshape(128, NP * KT * 128)).astype(ml_dtypes.bfloat16)
            in_maps.append({
                "xt": xt_sw,
                "wcat": w_sw,
                "bias0": np.ascontiguousarray(
                    decay_bias[c0:c0 + 128, None]),
                "bias1": np.ascontiguousarray(
                    decay_bias[c0 + 128:c0 + 192, None]),
            })
    return in_maps


def kernel(x, Wa, Wi, Wv, decay_bias):
    global _CACHED_NC
    if _CACHED_NC is None:
        _CACHED_NC = _build_nc()
    nc = _CACHED_NC

    in_maps = _make_in_maps(x, Wa, Wi, Wv, decay_bias)
    res = run_bass_kernel_spmd(nc, in_maps, core_ids=list(range(8)))

    out = np.empty((B, S, DR), dtype=np.float32)
    for b in range(B):
        for j in range(2):
            core = 2 * b + j
            out[b, :, j * DC:(j + 1) * DC] = res.results[core]["out"].T
    return out


# revision 72
# speedup vs baseline: 1.0263x; 1.0013x over previous
"""Trainium2 Bass kernel for the Griffin-style gated linear recurrence.

Model (matching the jax reference, including its chunked-scan numerics):
    a = sigmoid(x @ Wa.T + decay_bias)
    i = sigmoid(x @ Wi.T)
    v = x @ Wv.T
    w = sqrt(max(1 - a*a, 1e-8)) * i * v
    chunked scan (chunk=64): cum_decay = prod of a within chunk;
    weighted = w / max(cum_decay, 1e-10); intra = cum_decay * cumsum(weighted);
    states = intra + cum_decay * carry.

This is algebraically the single global recurrence
    h[t] = a[t] * h[t-1] + min(1, cd[t]*1e10) * w[t]
with cd the within-chunk running product of a.  Here it is computed as
    u'  = (i * 1e10) * v                       (scalar_tensor_tensor)
    w'  = r * u',  r = sqrt(1 - a^2)           (ACT sqrt + Pool mult)
    cd  = scan: state = max(a*state, d1m)      (d1m = a at chunk starts)
    gw  = min(cd, 1e-10) * w'                  (scalar_tensor_tensor)
    h   = scan: state = a*state + gw
The max-reset works because cd in (0,1] and a <= 1; folding 1e10 into u'
makes the clamp a single fused min+mult.

Sharding: 4 batches x 2 channel-halves = 8 cores, no communication.
Per core 192 channels; weights are concatenated into 5 x 128-row matmul
passes (P0=Wa-lo, P1=Wi-lo, P2=Wv-lo, P3=[Wi-hi;Wa-hi], P4=[Wv-hi;pad]);
the hi-group decay sigmoid runs in place at partitions 64-127 and its
output is DMA-shifted to partitions 0-63 so the hi group stays aligned.
x and weights are bf16 (halves HBM traffic); PSUM and all vector-engine
tensors are fp32 (bf16/mixed scans and Pool casts measure much slower).

The sqrt(1-a^2) ops would thrash the ACT table against the sigmoids
(sqrt and sigmoid live in different activation-table sets, ~1.3us per
reload, and the Tile scheduler dispatches greedily), so sqrt phases are
*gated*: each flush's activations take their bias (the 1.0 in
sqrt(1 - m)) from a [P,1] tile produced by a Pool compare (a>=a) on the
trigger block's output, making the whole sqrt batch data-dependent on
pipeline progress.  PSUM-consuming ops (u') are emitted at high priority
so flush bursts never starve the PE of PSUM banks.
"""

import sys

if "/opt/trn_rl_repo" not in sys.path:
    sys.path.insert(0, "/opt/trn_rl_repo")

from contextlib import ExitStack, contextmanager


@contextmanager
def _null_ctx():
    yield

import ml_dtypes
import numpy as np

from concourse import bacc, mybir, tile
from concourse.bass_utils import run_bass_kernel_spmd

B, S = 4, 4096
DM, DR = 1024, 384
DC = DR // 2          # channels per core (192)
CH = 64               # scan chunk size
SB = 512              # sequence block per pipeline step
NB = S // SB
KT = DM // 128        # contraction tiles
NP = 5                # matmul passes per block (640 = 5*128 rows)
WROWS = NP * 128

F32 = mybir.dt.float32
BF16 = mybir.dt.bfloat16
AFT = mybir.ActivationFunctionType
OP = mybir.AluOpType

PASS_ORDER = (3, 0, 1, 4, 2)

# blocks after whose section the pending blocks (excluding that block)
# are flushed through the sqrt phase
FLUSH_AT = (1, 3, 5, 7)

_CACHED_NC = None


def _build_nc():
    nc = bacc.Bacc(trn_type="TRN2")

    # host-swizzled layouts: xt is [128, NB*KT*SB] with per-(partition,
    # block) contiguous 8KB runs, wcat is [128, NP*KT*128] with per-pass
    # contiguous 2KB runs — few large DMA descriptors instead of ~1K tiny
    # ones per transfer.
    xT = nc.dram_tensor("xt", [128, NB * KT * SB], BF16, kind="ExternalInput")
    wT = nc.dram_tensor("wcat", [128, NP * KT * 128], BF16,
                        kind="ExternalInput")
    bias0 = nc.dram_tensor("bias0", [128, 1], F32, kind="ExternalInput")
    bias1 = nc.dram_tensor("bias1", [64, 1], F32, kind="ExternalInput")
    out = nc.dram_tensor("out", [DC, S], F32, kind="ExternalOutput")

    with tile.TileContext(nc) as tc, ExitStack() as ctx:
        wp = ctx.enter_context(tc.tile_pool(name="wp", bufs=1))
        cp = ctx.enter_context(tc.tile_pool(name="cp", bufs=1))
        xp = ctx.enter_context(tc.tile_pool(name="xp", bufs=2))
        pp = ctx.enter_context(tc.tile_pool(name="pp", bufs=1, space="PSUM"))
        shp = ctx.enter_context(tc.tile_pool(name="shp", bufs=2))
        # pools whose tiles survive until the sqrt flush.  Depth matters:
        # if a pool hits capacity in the compile-time scheduler's
        # simulation, it slots unrelated work (sigmoids) into the sqrt
        # bursts and thrashes the activation table.
        ap = ctx.enter_context(tc.tile_pool(name="ap", bufs=8))
        up = ctx.enter_context(tc.tile_pool(name="up", bufs=8))
        cdp = ctx.enter_context(tc.tile_pool(name="cdp", bufs=6))
        rp = ctx.enter_context(tc.tile_pool(name="rp", bufs=8))
        mp = ctx.enter_context(tc.tile_pool(name="mp", bufs=8))
        # short-lived per-block tiles
        ip = ctx.enter_context(tc.tile_pool(name="ip", bufs=2))
        wwp = ctx.enter_context(tc.tile_pool(name="wwp", bufs=4))
        gwp = ctx.enter_context(tc.tile_pool(name="gwp", bufs=4))
        hp = ctx.enter_context(tc.tile_pool(name="hp", bufs=2))
        gtp = ctx.enter_context(tc.tile_pool(name="gtp", bufs=2))

        # --- constants -------------------------------------------------
        # weights split per pass so the first matmul only waits on its
        # slice; emission of the actual DMAs happens after block 0's x DMA
        # so x wins the sync-queue dispatch race.
        w_sb = wp.tile([128, NP, KT, 128], BF16, tag="w")

        b0 = cp.tile([128, 1], F32, tag="b0")
        nc.scalar.dma_start(b0[:], bias0[:, :])
        # hi-group bias lives at partitions 64-127 to match its PSUM input
        b1 = cp.tile([128, 1], F32, tag="b1")
        nc.scalar.dma_start(b1[64:128, :], bias1[:, :])

        # persistent d1m tiles: zero except chunk-start columns, refreshed
        # with a[:, ::64] every block.
        d1m0 = cp.tile([128, SB], F32, tag="d1m0")
        nc.vector.memset(d1m0[:], 0.0)
        d1m1 = cp.tile([64, SB], F32, tag="d1m1")
        nc.vector.memset(d1m1[:], 0.0)

        # --- main pipeline over sequence blocks ------------------------
        pend = []
        state = {"init0": 0.0, "init1": 0.0}

        def flush(blocks, last=False):
            """Run the sqrt phase + w/gw/h for `blocks`.  The sqrt batch
            is gated (via its bias tile) on the last flushed block's u
            tiles — i.e. it becomes ready exactly when the flush's own
            inputs are complete — and runs at high priority, so the whole
            batch executes contiguously (one activation-table round trip)
            without the greedy scheduler interleaving sigmoids."""
            if not blocks:
                return
            # m = a*a is produced in the block body (spread Pool load).
            # The gate ties the sqrt batch to the LAST flushed block's m:
            # Pool executes in order, so m_last done implies every m in
            # the batch is done — all sqrts become ready simultaneously
            # and run contiguously (one activation-table round trip).
            if last:
                # nothing follows the final sqrt batch on ACT, so no
                # gating is needed — let it run the moment m lands
                bias0_ap = bias1_ap = 1.0
            else:
                lm0, lm1 = blocks[-1][7], blocks[-1][8]
                gate = gtp.tile([128, 1], F32, tag="gate")
                with tc.high_priority():
                    nc.vector.tensor_tensor(
                        gate[0:64, :], lm1[:, 0:1], lm1[:, 0:1],
                        op=OP.is_ge)
                    nc.vector.tensor_tensor(
                        gate[64:128, :], lm0[64:128, 0:1],
                        lm0[64:128, 0:1], op=OP.is_ge)
                bias0_ap, bias1_ap = gate[:], gate[0:64, :]
            rs = []
            with tc.high_priority():
                for (jb, aj0, aj1, uj0, uj1, cj0, cj1, m0, m1) in blocks:
                    r0 = rp.tile([128, SB], BF16, tag="r0")
                    nc.scalar.activation(
                        r0[:], m0[:], AFT.Sqrt, bias=bias0_ap, scale=-1.0)
                    r1 = rp.tile([64, SB], BF16, tag="r1")
                    nc.scalar.activation(
                        r1[:], m1[:], AFT.Sqrt, bias=bias1_ap, scale=-1.0)
                    rs.append((r0, r1))
            for (jb, aj0, aj1, uj0, uj1, cj0, cj1, m0, m1), (r0, r1) \
                    in zip(blocks, rs):
                sj = jb * SB
                # w0 on DVE (bf16 2x mode, ~360ns) in parallel with w1 on
                # Pool — halves the serial w chain in the flush bursts
                w0 = wwp.tile([128, SB], BF16, tag="w0")
                nc.vector.tensor_mul(w0[:], r0[:], uj0[:])
                w1 = wwp.tile([64, SB], BF16, tag="w1")
                nc.gpsimd.tensor_mul(w1[:], r1[:], uj1[:])
                # gw = min(cd, 1e-10) * w'   (the 1e10 is folded into u')
                gw0 = gwp.tile([128, SB], F32, tag="gw0")
                nc.vector.scalar_tensor_tensor(
                    gw0[:], cj0[:], 1e-10, w0[:], op0=OP.min, op1=OP.mult)
                gw1 = gwp.tile([64, SB], F32, tag="gw1")
                nc.vector.scalar_tensor_tensor(
                    gw1[:], cj1[:], 1e-10, w1[:], op0=OP.min, op1=OP.mult)
                h0 = hp.tile([128, SB], F32, tag="h0")
                nc.vector.tensor_tensor_scan(
                    h0[:], aj0[:], gw0[:], state["init0"],
                    op0=OP.mult, op1=OP.add)
                h1 = hp.tile([64, SB], F32, tag="h1")
                nc.vector.tensor_tensor_scan(
                    h1[:], aj1[:], gw1[:], state["init1"],
                    op0=OP.mult, op1=OP.add)
                # round-robin output DMAs over three engine queues so the
                # 3.1MB of h traffic drains in parallel instead of
                # serializing (~11us) on the sync queue at the end; the
                # final block's pair is split across two queues
                if last and jb == blocks[-1][0]:
                    nc.sync.dma_start(out[0:128, sj:sj + SB], h0[:])
                    nc.scalar.dma_start(out[128:DC, sj:sj + SB], h1[:])
                else:
                    out_dma = (nc.sync.dma_start, nc.scalar.dma_start,
                               nc.gpsimd.dma_start)[jb % 3]
                    out_dma(out[0:128, sj:sj + SB], h0[:])
                    out_dma(out[128:DC, sj:sj + SB], h1[:])
                state["init0"] = h0[:, SB - 1:SB]
                state["init1"] = h1[:, SB - 1:SB]

        x_src = xT.rearrange("p (nb k s) -> p nb k s", nb=NB, k=KT)
        w_src = wT.rearrange("p (np k c) -> p np k c", np=NP, k=KT)

        for ib in range(NB):
            s0 = ib * SB

            x_sb = xp.tile([128, KT, SB], BF16, tag="x")
            if ib == 0:
                # startup is aggregate-DMA-bandwidth-bound: sequence the
                # transfers so only w-P3 + x[k0] (384KB) gate the first
                # matmul and the rest streams in behind it
                with tc.high_priority():
                    nc.scalar.dma_start(w_sb[:, 3], w_src[:, 3])
                    for k in range(0, KT, 2):
                        nc.sync.dma_start(
                            x_sb[:, k:k + 1], x_src[:, ib, k:k + 1])
                    for k in range(1, KT, 2):
                        nc.scalar.dma_start(
                            x_sb[:, k:k + 1], x_src[:, ib, k:k + 1])
                for p in (0, 1, 4, 2):
                    nc.gpsimd.dma_start(w_sb[:, p], w_src[:, p])
            else:
                # two-queue split: a 1MB block takes ~9us on one queue,
                # leaving no slack against the ~10us block period
                nc.sync.dma_start(x_sb[:, 0:4], x_src[:, ib, 0:4])
                nc.scalar.dma_start(x_sb[:, 4:8], x_src[:, ib, 4:8])

            z = {}
            for p in PASS_ORDER:
                zt = pp.tile([128, SB], F32, tag=f"z{p}")
                for k in range(KT):
                    nc.tensor.matmul(
                        zt[:],
                        w_sb[:, p, k, :],
                        x_sb[:, k, :],
                        start=(k == 0),
                        stop=(k == KT - 1),
                    )
                z[p] = zt

            # ---- ACT sigmoids (single table set); PSUM consumers run at
            # high priority so flush bursts never stall the PE ----
            a1hi = shp.tile([128, SB], F32, tag="a1hi")
            a1 = ap.tile([64, SB], F32, tag="a1")
            a0 = ap.tile([128, SB], F32, tag="a0")
            i0 = ip.tile([128, SB], BF16, tag="i0")
            i1 = ip.tile([64, SB], BF16, tag="i1")
            u0 = up.tile([128, SB], BF16, tag="u0")
            u1 = up.tile([64, SB], BF16, tag="u1")
            nc.scalar.activation(
                a1hi[64:128, :], z[3][64:128, :], AFT.Sigmoid,
                bias=b1[64:128, :])
            nc.sync.dma_start(a1[:], a1hi[64:128, :])
            nc.scalar.activation(a0[:], z[0][:], AFT.Sigmoid, bias=b0[:])
            nc.scalar.activation(i0[:], z[1][:], AFT.Sigmoid)
            nc.scalar.activation(i1[:], z[3][0:64, :], AFT.Sigmoid)
            with tc.high_priority():
                # u' = (i * 1e10) * v
                nc.vector.scalar_tensor_tensor(
                    u0[:], i0[:], 1e10, z[2][:], op0=OP.mult, op1=OP.mult)
                nc.vector.scalar_tensor_tensor(
                    u1[:], i1[:], 1e10, z[4][0:64, :],
                    op0=OP.mult, op1=OP.mult)

            # ---- Pool: d1m refresh + m = a*a ----
            nc.gpsimd.tensor_copy(d1m0[:, 0:SB:CH], a0[:, 0:SB:CH])
            nc.gpsimd.tensor_copy(d1m1[:, 0:SB:CH], a1[:, 0:SB:CH])
            m0 = mp.tile([128, SB], F32, tag="m0")
            m1 = mp.tile([64, SB], F32, tag="m1")
            with tc.high_priority():
                if ib == NB - 1:
                    # last block: m on DVE so the tail-critical sqrt
                    # doesn't queue behind flush w-ops on Pool
                    nc.vector.tensor_mul(m0[:], a0[:], a0[:])
                    nc.vector.tensor_mul(m1[:], a1[:], a1[:])
                else:
                    nc.gpsimd.tensor_mul(m0[:], a0[:], a0[:])
                    nc.gpsimd.tensor_mul(m1[:], a1[:], a1[:])

            # ---- cd scan with max-reset (DVE) ----
            cd0 = cdp.tile([128, SB], F32, tag="cd0")
            nc.vector.tensor_tensor_scan(
                cd0[:], a0[:], d1m0[:], 0.0, op0=OP.mult, op1=OP.max)
            cd1 = cdp.tile([64, SB], F32, tag="cd1")
            nc.vector.tensor_tensor_scan(
                cd1[:], a1[:], d1m1[:], 0.0, op0=OP.mult, op1=OP.max)

            pend.append((ib, a0, a1, u0, u1, cd0, cd1, m0, m1))

            if ib in FLUSH_AT:
                flush(pend[:-1])
                pend = pend[-1:]

        flush(pend, last=True)

    nc.finalize()
    return nc


def _make_in_maps(x, Wa, Wi, Wv, decay_bias):
    x = np.asarray(x, dtype=np.float32)
    Wa = np.asarray(Wa, dtype=np.float32)
    Wi = np.asarray(Wi, dtype=np.float32)
    Wv = np.asarray(Wv, dtype=np.float32)
    decay_bias = np.asarray(decay_bias, dtype=np.float32)

    in_maps = []
    for b in range(B):
        # xt_sw[p, ib, k, s'] = x[b, ib*SB+s', k*128+p]
        xt_sw = np.ascontiguousarray(
            x[b].reshape(NB, SB, KT, 128).transpose(3, 0, 2, 1)
            .reshape(128, NB * KT * SB)).astype(ml_dtypes.bfloat16)
        for j in range(2):
            c0 = j * DC
            wcat = np.concatenate([
                Wa[c0:c0 + 128],
                Wi[c0:c0 + 128],
                Wv[c0:c0 + 128],
                Wi[c0 + 128:c0 + 192],
                Wa[c0 + 128:c0 + 192],
                Wv[c0 + 128:c0 + 192],
                np.zeros((64, DM), np.float32),
            ])                                   # [640, DM]
            # w_sw[p, pi, k, c] = wcat[pi*128 + c, k*128 + p]
            w_sw = np.ascontiguousarray(
                wcat.reshape(NP, 128, KT, 128).transpose(3, 0, 2, 1)
                .reshape(128, NP * KT * 128)).astype(ml_dtypes.bfloat16)
            in_maps.append({
                "xt": xt_sw,
                "wcat": w_sw,
                "bias0": np.ascontiguousarray(
                    decay_bias[c0:c0 + 128, None]),
                "bias1": np.ascontiguousarray(
                    decay_bias[c0 + 128:c0 + 192, None]),
            })
    return in_maps


def kernel(x, Wa, Wi, Wv, decay_bias):
    global _CACHED_NC
    if _CACHED_NC is None:
        _CACHED_NC = _build_nc()
    nc = _CACHED_NC

    in_maps = _make_in_maps(x, Wa, Wi, Wv, decay_bias)
    res = run_bass_kernel_spmd(nc, in_maps, core_ids=list(range(8)))

    out = np.empty((B, S, DR), dtype=np.float32)
    for b in range(B):
        for j in range(2):
            core = 2 * b + j
            out[b, :, j * DC:(j + 1) * DC] = res.results[core]["out"].T
    return out
